# revision 42
# baseline (speedup 1.0000x reference)
"""Trainium2 Bass kernel for nn_ActivationQuantizer (quantize + im2col + topk row/col masking).

Pipeline (8 NeuronCores, data-parallel over batch B=8, one image per core):
  Host:     global min/max (2-scalar reduction) -> scale, exact zero boundary X0.
  Launch B: per-core nonzero-count stats (row sums, col-sum tree-fold,
            corners, per-pixel channel-sum map via ones-matmul)
            -> host: all-reduce row counts, 3x3 box-sum col counts,
               sort -> thresholds r1, r2 (the cross-device "all-reduce
               then threshold" step from the sharding hint).
  Launch C: per-core quantize + 9-shift im2col expansion with row/col
            masks folded into one scalar_tensor_tensor per plane slice,
            writes [1152, 3136] f32 at ~HBM write rate.
  Host:     interleave per-core outputs into [1152, 25088] (batch-minor).

Exactness strategy: the row/col masks depend on integer nonzero counts of
q = round(x/scale). round(t)==0 <=> |t| <= 0.5 (RNE), and f32 division is
monotone, so q!=0 <=> |x| > X0 where X0 = largest f32 with fl(X0/scale) <= 0.5
(found on host by exact f32 search). The device tests |x| > X0 with exact
comparisons, so counts match the jax reference bit-exactly. Output q values use
the f32 magic-number RNE trick (x*inv + M) - M; an off-by-one ULP there only
perturbs a handful of element values by ~scale, never the masks.
"""

import sys

if "/opt/trn_rl_repo" not in sys.path:
    sys.path.insert(0, "/opt/trn_rl_repo")

import math

import ml_dtypes
import numpy as np

import concourse.bacc as bacc
import concourse.mybir as mybir
from concourse.tile import TileContext
from concourse.bass_utils import run_bass_kernel_spmd

F32 = mybir.dt.float32
BF16 = mybir.dt.bfloat16
ALU = mybir.AluOpType
AX = mybir.AxisListType

B, C, H, W = 8, 128, 56, 56
HW = H * W              # 3136
PH, PW = H + 2, W + 2   # 58
PHW = PH * PW           # 3364
NO = 9                  # 3x3 filter offsets
R = C * NO              # 1152 output rows
L = B * HW              # 25088 output cols
RATIO = (0.2, 0.2)
MAGIC = float(np.float32(12582912.0))  # 1.5 * 2**23: f32 RNE rounding constant

CORES = list(range(8))

_NC_CACHE = {}

LAST_PROFILE = {}


def _nc_counts():
    nc = bacc.Bacc()
    x = nc.dram_tensor("x", [C, HW], F32, kind="ExternalInput")
    thr = nc.dram_tensor("thr", [C, 1], F32, kind="ExternalInput")
    # stats layout per channel: RS[0:56] | CS[56:112] | q00,q05,q50,q55 [112:116] | T [116]
    stats = nc.dram_tensor("stats", [C, 117], F32, kind="ExternalOutput")
    smap = nc.dram_tensor("smap", [1, HW], F32, kind="ExternalOutput")
    with TileContext(nc) as tc:
        with (
            tc.tile_pool(name="p", bufs=1) as pool,
            tc.tile_pool(name="ps", bufs=4, space="PSUM") as psp,
        ):
            xt = pool.tile([C, HW], F32)
            th = pool.tile([C, 1], F32)
            nc.sync.dma_start(out=th[:, :], in_=thr[:, :])
            absx = pool.tile([C, HW], F32)
            nzb = pool.tile([C, HW], BF16)
            st = pool.tile([C, 117], F32)
            nz3 = nzb[:, :].rearrange("c (h w) -> c h w", h=H)
            NCH = 4
            RCH = H // NCH  # 14 rows per chunk
            CH = RCH * W
            for j in range(NCH):
                sl = slice(j * CH, (j + 1) * CH)
                eng = nc.sync if j % 2 == 0 else nc.scalar
                eng.dma_start(out=xt[:, sl], in_=x[:, sl])
                # nz = (|x| > X0) as bf16 0/1 (exact):
                # |x| via sign-bit clear on the int32 view, then compare
                nc.vector.tensor_scalar(
                    absx[:, sl].bitcast(mybir.dt.uint32),
                    xt[:, sl].bitcast(mybir.dt.uint32),
                    0x7FFFFFFF,
                    None,
                    ALU.bitwise_and,
                )
                nc.vector.tensor_scalar(
                    nzb[:, sl], absx[:, sl], th[:, 0:1], None, ALU.is_gt
                )
                nc.vector.tensor_reduce(
                    st[:, j * RCH : (j + 1) * RCH],
                    nz3[:, j * RCH : (j + 1) * RCH, :],
                    axis=AX.X,
                    op=ALU.add,
                )
            # CS[c,w] = sum_h nz[c,h,w]: contiguous tree-fold over h rows
            # (56 = 8*7): fold 28+28, 14+14, 7+7 -> [7,56], then reduce the
            # 7 rows via a strided-X reduce over a small [c,56,7] view.
            fold = pool.tile([C, 28 * W], F32)
            nc.vector.tensor_tensor(
                fold[:, : 28 * W], nzb[:, : 28 * W], nzb[:, 28 * W :], ALU.add
            )
            nc.vector.tensor_tensor(
                fold[:, : 14 * W], fold[:, : 14 * W], fold[:, 14 * W : 28 * W], ALU.add
            )
            nc.vector.tensor_tensor(
                fold[:, : 7 * W], fold[:, : 7 * W], fold[:, 7 * W : 14 * W], ALU.add
            )
            f7 = fold[:, : 7 * W].rearrange("c (h w) -> c w h", h=7)
            nc.vector.tensor_reduce(st[:, 56:112], f7, axis=AX.X, op=ALU.add)
            nc.vector.tensor_copy(st[:, 112:114], nzb[:, 0 : W : W - 1])
            nc.vector.tensor_copy(st[:, 114:116], nzb[:, (H - 1) * W : HW : W - 1])
            nc.vector.tensor_reduce(st[:, 116:117], st[:, 0:56], axis=AX.X, op=ALU.add)
            # channel-sum map S[hw] = sum_c nz[c, hw] via ones-matmul (PSUM 512/bank)
            ones = pool.tile([C, 1], BF16)
            nc.vector.memset(ones[:, :], 1.0)
            ssb = pool.tile([1, HW], F32)
            nchunk = (HW + 511) // 512
            for j in range(nchunk):
                n = min(512, HW - j * 512)
                pt = psp.tile([1, 512], F32, tag="pt")
                nc.tensor.matmul(
                    pt[0:1, 0:n],
                    ones[:, 0:1],
                    nzb[:, j * 512 : j * 512 + n],
                    start=True,
                    stop=True,
                )
                nc.scalar.copy(ssb[0:1, j * 512 : j * 512 + n], pt[0:1, 0:n])
            nc.sync.dma_start(out=stats[:, :], in_=st[:, :])
            nc.sync.dma_start(out=smap[:, :], in_=ssb[0:1, :])
    nc.compile()
    return nc


def _nc_expand():
    nc = bacc.Bacc()
    x = nc.dram_tensor("x", [C, HW], F32, kind="ExternalInput")
    inv = nc.dram_tensor("inv", [C, 1], F32, kind="ExternalInput")
    rs9 = nc.dram_tensor("rs9", [C, NO], F32, kind="ExternalInput")
    cm = nc.dram_tensor("cm", [1, HW], BF16, kind="ExternalInput")
    out = nc.dram_tensor("out", [R, HW], F32, kind="ExternalOutput")
    outv = out[:, :].rearrange("(c o) l -> c o l", o=NO)
    with TileContext(nc) as tc:
        with (
            tc.tile_pool(name="p", bufs=1) as pool,
            tc.tile_pool(name="pp", bufs=5) as pp,
            tc.tile_pool(name="ps", bufs=4, space="PSUM") as psp,
        ):
            xt = pool.tile([C, HW], F32)
            # uneven split: chunk 1 = rows [0,30) so the first half-planes
            # (which need qp rows [0,30)) unblock as early as possible
            RSPLIT = 30
            nc.sync.dma_start(out=xt[:, : 15 * W], in_=x[:, : 15 * W])
            nc.scalar.dma_start(
                out=xt[:, 15 * W : RSPLIT * W], in_=x[:, 15 * W : RSPLIT * W]
            )
            nc.sync.dma_start(
                out=xt[:, RSPLIT * W : 43 * W], in_=x[:, RSPLIT * W : 43 * W]
            )
            nc.scalar.dma_start(out=xt[:, 43 * W :], in_=x[:, 43 * W :])
            invt = pool.tile([C, 1], F32)
            nc.sync.dma_start(out=invt[:, :], in_=inv[:, :])
            rst = pool.tile([C, NO], F32)
            nc.sync.dma_start(out=rst[:, :], in_=rs9[:, :])
            # broadcast cm to all partitions via a K=1 bf16 matmul (PE + ACT
            # are otherwise idle; keeps the DMA queues free for x / output)
            cmsrc = pool.tile([1, HW], BF16)
            nc.sync.dma_start(out=cmsrc[:, :], in_=cm[:, :])
            onesb = pool.tile([1, C], BF16)
            nc.vector.memset(onesb[:, :], 1.0)
            cmt = pool.tile([C, HW], F32)
            nchunk = (HW + 511) // 512
            for j in range(nchunk):
                n = min(512, HW - j * 512)
                pcm = psp.tile([C, 512], F32, tag="pcm")
                nc.tensor.matmul(
                    pcm[:, 0:n],
                    onesb[0:1, :],
                    cmsrc[0:1, j * 512 : j * 512 + n],
                    start=True,
                    stop=True,
                )
                nc.scalar.copy(cmt[:, j * 512 : j * 512 + n], pcm[:, 0:n])
            cm3 = cmt[:, :].rearrange("c (h w) -> c h w", h=H)
            # padded quantized image qp[c, 58, 58]; zero only the border ring
            qp = pool.tile([C, PHW], F32)
            qv = qp[:, :].rearrange("c (a b) -> c a b", a=PH)
            nc.vector.memset(qv[:, 0, :], 0.0)
            nc.vector.memset(qv[:, PH - 1, :], 0.0)
            nc.vector.memset(qv[:, 1 : PH - 1, 0], 0.0)
            nc.vector.memset(qv[:, 1 : PH - 1, PW - 1], 0.0)
            qpi = qv[:, 1 : 1 + H, 1 : 1 + W]
            # q = RNE(x * inv) via magic add/sub; linear intermediate, and
            # computed in two row-chunks matching the x load split
            ql = pool.tile([C, HW], F32)
            ql3 = ql[:, :].rearrange("c (h w) -> c h w", h=H)

            def q_rows(r0, r1):
                sl = slice(r0 * W, r1 * W)
                nc.vector.tensor_scalar(
                    ql[:, sl], xt[:, sl], invt[:, 0:1], MAGIC, ALU.mult, ALU.add
                )
                nc.vector.tensor_scalar(
                    qpi[:, r0:r1, :], ql3[:, r0:r1, :], 1.0, -MAGIC,
                    ALU.mult, ALU.add,
                )

            HALF = H // 2
            ne = 0

            def plane_part(o, r0, nr):
                nonlocal ne
                fi, fj = divmod(o, 3)
                pl = pp.tile([C, nr * W], F32, tag="pl", name=f"pl{o}_{r0}")
                pl3 = pl[:, :].rearrange("c (h w) -> c h w", h=nr)
                qs = qv[:, fi + r0 : fi + r0 + nr, fj : fj + W]
                nc.vector.scalar_tensor_tensor(
                    pl3,
                    qs,
                    rst[:, o : o + 1],
                    cm3[:, r0 : r0 + nr, :],
                    ALU.mult,
                    ALU.mult,
                )
                eng = nc.sync if ne % 2 == 0 else nc.scalar
                ne += 1
                eng.dma_start(
                    out=outv[:, o, r0 * W : (r0 + nr) * W], in_=pl[:, :]
                )

            q_rows(0, RSPLIT)          # qp rows [1,31) ready
            for o in range(4):         # top halves need qp rows [0,30)
                plane_part(o, 0, HALF)
            q_rows(RSPLIT, H)          # qp rows [31,57) ready
            for o in range(4):
                plane_part(o, HALF, H - HALF)
            for o in range(4, NO):
                plane_part(o, 0, H)
    nc.compile()
    return nc


def _get(name, builder):
    if name not in _NC_CACHE:
        _NC_CACHE[name] = builder()
    return _NC_CACHE[name]


def _run(nc, in_maps, **kw):
    """run_bass_kernel_spmd with one retry (transient device-wedge insurance)."""
    try:
        return run_bass_kernel_spmd(nc, in_maps, core_ids=CORES, **kw)
    except Exception:
        import time

        time.sleep(2.0)
        return run_bass_kernel_spmd(nc, in_maps, core_ids=CORES, **kw)


def _find_x0(scale):
    """Largest f32 v with fl(v/scale) <= 0.5 (q==0 boundary under RNE)."""
    s = np.float32(scale)
    half = np.float32(0.5)
    v = np.float32(half * s)
    inf32 = np.float32(np.inf)
    while np.float32(v) / s > half:
        v = np.nextafter(v, -inf32, dtype=np.float32)
    while True:
        nv = np.nextafter(v, inf32, dtype=np.float32)
        if np.float32(nv) / s <= half:
            v = nv
        else:
            break
    return np.float32(v)


def kernel(x, bits, _trace=False):
    bits = int(bits)
    x = np.ascontiguousarray(np.asarray(x, dtype=np.float32))
    assert x.shape == (B, C, H, W), x.shape
    xb = x.reshape(B, C, HW)

    trace_kw = {"trace": True} if _trace else {}
    LAST_PROFILE.clear()

    # ---- global min/max (2-scalar reduction, host) -> scale, X0 ----
    mn = np.float32(np.min(x))
    mx = np.float32(np.max(x))
    scale = np.float32((mx - mn) / np.float32(2**bits - 1))
    inv_scale = np.float32(np.float32(1.0) / scale)
    x0 = _find_x0(scale)

    # ---- Launch B: nonzero-count stats ----
    ncB = _get("counts", _nc_counts)
    thr = np.full((C, 1), x0, dtype=np.float32)
    resB = _run(ncB, [{"x": xb[b], "thr": thr} for b in range(B)], **trace_kw)
    if _trace:
        LAST_PROFILE["B_ns"] = resB.exec_time_ns

    # host: per-core row counts nzr_b[c, fi, fj] and col counts nzc_b[oi, oj]
    nzr = np.zeros((C, 3, 3), dtype=np.int64)
    nzc_per_core = []
    for b in range(B):
        st = resB.results[b]["stats"].astype(np.float64)
        RS = st[:, 0:56]
        CS = st[:, 56:112]
        q00, q05 = st[:, 112], st[:, 113]
        q50, q55 = st[:, 114], st[:, 115]
        T = st[:, 116]
        row_excl = [RS[:, 55], np.zeros(C), RS[:, 0]]   # fi = 0,1,2
        col_excl = [CS[:, 55], np.zeros(C), CS[:, 0]]   # fj = 0,1,2
        corner = {
            (0, 0): q55, (0, 2): q50,
            (2, 0): q05, (2, 2): q00,
        }
        for fi in range(3):
            for fj in range(3):
                v = T - row_excl[fi] - col_excl[fj] + corner.get((fi, fj), 0.0)
                nzr[:, fi, fj] += np.rint(v).astype(np.int64)
        S = resB.results[b]["smap"].reshape(H, W).astype(np.float64)
        Sp = np.pad(S, 1)
        nzc = np.zeros((H, W), dtype=np.float64)
        for di in range(3):
            for dj in range(3):
                nzc += Sp[di : di + H, dj : dj + W]
        nzc_per_core.append(np.rint(nzc).astype(np.int64).reshape(HW))

    nzr_flat = nzr.reshape(R)  # r = c*9 + fi*3 + fj
    r1 = np.sort(nzr_flat)[int(math.ceil(R * RATIO[0]))]
    nzc_all = np.concatenate(nzc_per_core)
    r2 = np.sort(nzc_all)[int(math.ceil(L * RATIO[1]))]

    rowscale = np.where(nzr_flat >= r1, scale, np.float32(0.0)).astype(np.float32)
    rs9 = np.ascontiguousarray(rowscale.reshape(C, NO))
    invrep = np.full((C, 1), inv_scale, dtype=np.float32)

    # ---- Launch C: masked im2col expansion ----
    ncC = _get("expand", _nc_expand)
    in_maps = []
    for b in range(B):
        cm_b = (
            (nzc_per_core[b] >= r2)
            .astype(ml_dtypes.bfloat16)
            .reshape(1, HW)
        )
        in_maps.append({"x": xb[b], "inv": invrep, "rs9": rs9, "cm": cm_b})
    resC = _run(ncC, in_maps, **trace_kw)
    if _trace:
        LAST_PROFILE["C_ns"] = resC.exec_time_ns

    outs = [resC.results[b]["out"] for b in range(B)]  # each [R, HW]
    full = np.stack(outs, axis=2).reshape(R, L)
    return full
